# revision 28
# baseline (speedup 1.0000x reference)
"""Trainium2 Bass kernel for causal multi-head attention with RoPE.

nn_CausalAttention: x [2, 2048, 2048], Wq/Wk/Wv [2048, 2048] (y = x @ W.T),
16 heads of dim 128, RoPE, causal fp32 softmax.

Sharding: core = (batch, head-group-of-4). Each of the 8 NeuronCores owns
one batch element and 4 heads (a 512-wide slice of the QKV output dim). The
full output is assembled on host by concatenating per-core feature slices (no
collectives needed).

Per-core kernel (Bass/Tile, bf16 matmul operands / fp32 accumulation+softmax,
measured end-to-end rel err ~4e-3 vs the 2e-2 gate; fp8 was measured to
break the gate and bf16 streams the PE at the same 1 col/cycle as fp32r):
  Fused pipeline over 4 sequence slabs of 512. For slab e:
    Phase A(e): q^T/k^T in [head_dim x seq] layout (weights stationary, x^T
      moving 512-wide); RoPE fused into the PSUM->SBUF eviction via the
      quadrant-16 row-permuted weight layout (pair-combine is one DVE
      stream_shuffle). v is projected directly into [seq x head_dim] layout
      (x^T tiles stationary, Wv^T moving) - no PE transposes; the psum
      eviction cast runs on the otherwise-idle scalar engine.
    Phase B(e): causal attention for query tile qt=e of all 4 heads in
      transposed-score layout S^T = K-tile^T^T @ q^T, diagonal tiles
      emitted first so accumulation-group start flags land on full-width
      matmuls and round-end drains wait on long-finished exps. exp on the
      scalar engine with the 1/sqrt(d) scale fused; diagonal tiles
      sub-ranged with a 128x128 triangular mask post-exp. Softmax
      denominator: full key tiles pair- then quad-summed on DVE (bf16),
      one ones-matmul per quad broadcasts the cross-partition sum into
      PSUM (1/4 the PE work of per-tile den matmuls). Normalization fused
      into the eviction with a fast approximate reciprocal, split in
      half-tiles to pipeline recip->mul->DMA; output is head-dim-major,
      untransposed on the host during the gather.
  Emitting B(e) between A(e) and A(e+1) hides the exp stream and DVE work
  under the next slab's projection matmuls. The last round (no slab behind
  it) switches to 2-bank score tiles - phase A's PSUM banks are free by
  then - with one exp per key-tile pair so the scalar engine outruns the PE.
Startup: all inputs are host-repacked partition-major so every DMA reads
multi-KB contiguous lines; slab-0 interleaves x/W blocks in consumption
order; ~50 dummy matmuls on a memset tile warm the PE clock-gate (HAM)
during the initial DMA window.
"""

import math

import numpy as np
import ml_dtypes

import concourse.bacc as bacc
import concourse.bass as bass
import concourse.mybir as mybir
import concourse.tile as tile
from concourse import bass_utils

F32 = mybir.dt.float32
BF16 = mybir.dt.bfloat16
AF = mybir.ActivationFunctionType

S = 2048
M = 2048
NCORES = 8

D = 128          # head dim
NH = 4           # heads per core
SLAB = 512       # sequence slab (phase A) == query tile (phase B)
NE = S // SLAB
MC = M // 128
QT = SLAB


def _rope_perm(n):
    """Row permutation for the quadrant-16 RoPE layout.

    New row p (within a 128-row head block): quadrant qd = p//32, r = p%32.
    r < 16  -> even element of pair i = 16*qd + r      (old row 2i)
    r >= 16 -> odd  element of pair i = 16*qd + (r-16) (old row 2i+1)
    Pair elements are 16 partitions apart inside one 32-partition quadrant,
    so the RoPE combine is a stream_shuffle with a 16-rotation mask.
    """
    perm = []
    for hb in range(n // D):
        base = hb * D
        for qd in range(4):
            perm += [base + 2 * (16 * qd + r) for r in range(16)]
            perm += [base + 2 * (16 * qd + r) + 1 for r in range(16)]
    return np.array(perm)


SWAP16 = [(i + 16) % 32 for i in range(32)]


def prep_core_inputs(x, Wq, Wk, Wv, core, S, M):
    """Host-side shard prep for one core: batch b = core//4, heads
    [hg*4, hg*4+4) where hg = core%4 (rows [hg*512, hg*512+512) of W*)."""
    b, hg = core // 4, core % 4
    nsl = slice(hg * NH * D, (hg + 1) * NH * D)
    perm = _rope_perm(NH * D)
    wq = Wq[nsl][perm]
    wk = Wk[nsl][perm]
    wv = Wv[nsl]

    def bf(a):
        return np.ascontiguousarray(a).astype(ml_dtypes.bfloat16)

    # Partition-major repacks so every DMA reads large contiguous lines
    # per partition (host prep is not on the measured clock):
    #   xTp[e, p, mo, s] = x[b].T[mo*128+p, e*SLAB+s]
    #   w*p[hp, p, mo, c] = W*.T[mo*128+p, hp*256+c]   (head-pair-major)
    #   wvp[p, mo, c]    = Wv.T[mo*128+p, c]
    xT = x[b].T
    xTp = xT.reshape(MC, 128, NE, SLAB).transpose(2, 1, 0, 3)
    wqp = wq.T.reshape(MC, 128, 2, 2 * D).transpose(2, 1, 0, 3)
    wkp = wk.T.reshape(MC, 128, 2, 2 * D).transpose(2, 1, 0, 3)
    wvp = wv.T.reshape(MC, 128, NH * D).transpose(1, 0, 2)

    theta = np.exp(
        -np.float32(np.log(10000.0))
        * (np.arange(0, D, 2, dtype=np.float32) / np.float32(D))
    ).astype(np.float32)
    pos = np.arange(S, dtype=np.float32)
    freqs = theta[:, None] * pos[None, :]  # [64, S]
    cos_t, sin_t = np.cos(freqs), np.sin(freqs)
    # quadrant-16 layout: partition p -> pair i(p) = 16*(p//32) + (p%16)
    p = np.arange(128)
    i_of_p = 16 * (p // 32) + (p % 16)
    is_odd = (p % 32) >= 16
    packC = cos_t[i_of_p].astype(np.float32)                    # [128, S]
    packS = np.where(
        is_odd[:, None], -sin_t[i_of_p], sin_t[i_of_p]
    ).astype(np.float32)

    kk, qq = np.meshgrid(np.arange(128), np.arange(128), indexing="ij")
    tri = (kk <= qq).astype(np.float32)

    return {
        "xTp": bf(xTp),            # [NE, 128, MC, SLAB]
        "wqp": bf(wqp),            # [2, 128, MC, 2*D]
        "wkp": bf(wkp),
        "wvp": bf(wvp),            # [128, MC, NH*D]
        "packC": packC.astype(ml_dtypes.bfloat16),
        "packS": packS.astype(ml_dtypes.bfloat16),
        "tri": bf(tri),
        "ones": bf(np.ones((128, 128), dtype=np.float32)),
    }


def build_attention(tc: tile.TileContext, io: dict):
    nc = tc.nc
    scale = 1.0 / math.sqrt(D)
    outT = io["outT"]

    with (
        tc.tile_pool(name="wpool", bufs=1) as wpool,
        tc.tile_pool(name="qkvp", bufs=1) as qkvp,
        tc.tile_pool(name="packp", bufs=1) as packp,
        tc.tile_pool(name="constp", bufs=1) as constp,
        tc.tile_pool(name="xp", bufs=2) as xpool,
        tc.tile_pool(name="rope", bufs=2) as ropep,
        tc.tile_pool(name="expp", bufs=8) as expp,
        tc.tile_pool(name="dp", bufs=2) as dpool,
        tc.tile_pool(name="outp", bufs=2) as outp,
    ):
        wq_sb = wpool.tile([128, 2, MC, 2 * D], BF16, name="wq_sb")
        wk_sb = wpool.tile([128, 2, MC, 2 * D], BF16, name="wk_sb")
        wv_sb = wpool.tile([128, MC, NH * D], BF16, name="wv_sb")
        qT_sb = qkvp.tile([128, NH, S], BF16, name="qT_sb")
        kT_sb = qkvp.tile([128, NH, S], BF16, name="kT_sb")
        v_sb = qkvp.tile([128, S // 128, NH * D], BF16, name="v_sb")
        packC = packp.tile([128, S], BF16, name="packC")
        packS = packp.tile([128, S], BF16, name="packS")
        tri_sb = constp.tile([128, 128], BF16, name="tri_sb")
        ones_sb = constp.tile([128, 128], BF16, name="ones_sb")
        warm = constp.tile([128, 256], BF16, name="warm")

        xe_tiles = {}

        def get_xe(e):
            if e not in xe_tiles:
                xe_tiles[e] = xpool.tile(
                    [128, MC, SLAB], BF16, tag="xe", name=f"xe{e}"
                )
            return xe_tiles[e]

        def emit_b(e, psSp, psOp, psDp, paired):
            """Phase B round: causal attention for query tile qt=e, all
            heads, in transposed-score layout. paired=True packs two key
            tiles into one 2-bank PSUM score tile with a single exp per
            pair (used for the last round, where the exp stream would
            otherwise outpace the PE). The softmax denominator uses bf16
            pair- then quad-sums on DVE, with one ones-matmul per quad."""
            qt = e
            nkt = 4 * (e + 1)
            nfull = nkt - 4
            for h in range(NH):
                out_ps = psOp.tile([128, QT], F32, tag="o", name="out_ps")
                den_ps = psDp.tile([128, QT], F32, tag="d", name="den_ps")
                # start/stop by emission order (diagonal tiles go first, so
                # the first matmul of each accumulation group is full-width)
                n_den = 4 + bin(nfull // 4).count("1")
                den_left = [n_den]
                av_left = [nkt]
                pend = []
                dpairs = []

                def dmm(src, rs, _unused=None):
                    nc.tensor.matmul(
                        den_ps[:, rs:], ones_sb[:], src,
                        start=(den_left[0] == n_den),
                        stop=(den_left[0] == 1),
                    )
                    den_left[0] -= 1

                def av(src, kt, rs):
                    nc.tensor.matmul(
                        out_ps[:, rs:], v_sb[:, kt, h * D:(h + 1) * D], src,
                        start=(av_left[0] == nkt), stop=(av_left[0] == 1),
                    )
                    av_left[0] -= 1

                def drain(n):
                    while len(pend) > n:
                        for f in pend.pop(0):
                            f()

                dquads = []

                def add_pair(src0, src1, ops):
                    dpair = dpool.tile([128, QT], BF16, tag="dp",
                                       name="dpair")
                    nc.vector.tensor_add(dpair[:], src0, src1)
                    dpairs.append(dpair)
                    if len(dpairs) == 2:
                        a, b_ = dpairs
                        dpairs.clear()
                        dq = dpool.tile([128, QT], BF16, tag="dq",
                                        name="dquad")
                        nc.vector.tensor_add(dq[:], a[:], b_[:])
                        dquads.append(dq)
                        if len(dquads) == 2:
                            c, d_ = dquads
                            dquads.clear()
                            do = dpool.tile([128, QT], BF16, tag="do",
                                            name="doct")
                            nc.vector.tensor_add(do[:], c[:], d_[:])
                            dquads.append(do)
                            dquads.append(None)  # marker: slot 0 is an oct

                def flush_den():
                    for t in dquads:
                        if t is not None:
                            dmm(t[:], 0)
                    dquads.clear()

                if not paired:
                    held = {}
                    for kt in list(range(nfull, nkt)) + list(range(nfull)):
                        j = kt - nfull
                        rs = 128 * j if j > 0 else 0
                        s_ps = psSp.tile([128, QT], F32, tag="s", name="s_ps")
                        nc.tensor.matmul(
                            s_ps[:, rs:],
                            kT_sb[:, h, kt * 128:(kt + 1) * 128],
                            qT_sb[:, h, qt * QT + rs:(qt + 1) * QT],
                            start=True, stop=True,
                        )
                        expS = expp.tile([128, QT], BF16, tag="exp",
                                         name="expS")
                        nc.scalar.activation(
                            expS[:, rs:], s_ps[:, rs:], AF.Exp, scale=scale
                        )
                        if j >= 0:
                            nc.vector.tensor_mul(
                                expS[:, 128 * j:128 * (j + 1)],
                                expS[:, 128 * j:128 * (j + 1)],
                                tri_sb[:],
                            )
                        ops = []
                        if kt < nfull:
                            if kt % 2 == 0:
                                held[kt] = expS
                            else:
                                add_pair(held.pop(kt - 1)[:], expS[:], ops)
                        else:
                            ops.append(
                                lambda expS=expS, rs=rs, kt=kt:
                                dmm(expS[:, rs:], rs)
                            )
                        ops.append(
                            lambda expS=expS, kt=kt, rs=rs:
                            av(expS[:, rs:], kt, rs)
                        )
                        pend.append(ops)
                        drain(3)
                    flush_den()
                    drain(0)
                else:
                    npair = nkt // 2
                    for p in list(range(nfull // 2, npair)) + list(range(nfull // 2)):
                        k0, k1 = 2 * p, 2 * p + 1
                        j0, j1 = k0 - nfull, k1 - nfull
                        rs0 = 128 * j0 if j0 > 0 else 0
                        rs1 = 128 * j1 if j1 > 0 else 0
                        sp = psSp.tile([128, 2 * QT], F32, tag="s2",
                                       name="sp")
                        nc.tensor.matmul(
                            sp[:, rs0:QT],
                            kT_sb[:, h, k0 * 128:(k0 + 1) * 128],
                            qT_sb[:, h, qt * QT + rs0:(qt + 1) * QT],
                            start=True, stop=True,
                        )
                        nc.tensor.matmul(
                            sp[:, QT + rs1:],
                            kT_sb[:, h, k1 * 128:(k1 + 1) * 128],
                            qT_sb[:, h, qt * QT + rs1:(qt + 1) * QT],
                            start=True, stop=True,
                        )
                        expS = expp.tile([128, 2 * QT], BF16, tag="exp2",
                                         name="expS2")
                        if j1 <= 0:
                            nc.scalar.activation(
                                expS[:, rs0:], sp[:, rs0:], AF.Exp,
                                scale=scale,
                            )
                        else:
                            # diagonal pair: skip the stale sub-range the
                            # second score matmul left unwritten
                            nc.scalar.activation(
                                expS[:, rs0:QT], sp[:, rs0:QT], AF.Exp,
                                scale=scale,
                            )
                            nc.scalar.activation(
                                expS[:, QT + rs1:], sp[:, QT + rs1:], AF.Exp,
                                scale=scale,
                            )
                        for i, j in ((0, j0), (1, j1)):
                            if j >= 0:
                                cs = slice(i * QT + 128 * j,
                                           i * QT + 128 * (j + 1))
                                nc.vector.tensor_mul(
                                    expS[:, cs], expS[:, cs], tri_sb[:]
                                )
                        ops = []
                        if k1 < nfull:
                            add_pair(expS[:, 0:QT], expS[:, QT:], ops)
                        else:
                            ops.append(
                                lambda expS=expS, rs0=rs0:
                                dmm(expS[:, rs0:QT], rs0)
                            )
                            ops.append(
                                lambda expS=expS, rs1=rs1:
                                dmm(expS[:, QT + rs1:], rs1)
                            )
                        ops.append(
                            lambda expS=expS, k0=k0, rs0=rs0:
                            av(expS[:, rs0:QT], k0, rs0)
                        )
                        ops.append(
                            lambda expS=expS, k1=k1, rs1=rs1:
                            av(expS[:, QT + rs1:], k1, rs1)
                        )
                        pend.append(ops)
                        drain(2)
                    flush_den()
                    drain(0)

                # normalize+store in two half-tiles so the recip->mul->DMA
                # chain pipelines (shortens the end-of-kernel drain)
                for hf in range(2):
                    cs = slice(hf * (QT // 2), (hf + 1) * (QT // 2))
                    recip = outp.tile([128, QT // 2], F32, tag="recip",
                                      name="recip")
                    nc.vector.reciprocal_approx_fast(recip[:], den_ps[:, cs])
                    o_sb = outp.tile([128, QT // 2], F32, tag="o",
                                     name="o_sb")
                    nc.vector.tensor_mul(o_sb[:], out_ps[:, cs], recip[:])
                    nc.sync.dma_start(
                        outT[h, :, qt * QT + hf * (QT // 2):
                             qt * QT + (hf + 1) * (QT // 2)],
                        o_sb[:],
                    )

        with (
            tc.tile_pool(name="psA", bufs=4, space="PSUM") as psA,
            tc.tile_pool(name="psS", bufs=2, space="PSUM") as psS,
            tc.tile_pool(name="psO", bufs=1, space="PSUM") as psO,
            tc.tile_pool(name="psD", bufs=1, space="PSUM") as psD,
        ):
            # HAM prewarm: the PE clock-gate needs ~3.4us of sustained
            # activity to unthrottle from 1.2 to 2.4 GHz. The first input
            # DMAs take ~10us to land, so spend that dead window on dummy
            # matmuls over a memset tile (no DMA dependency) and the real
            # matmuls start at full clock.
            nc.vector.memset(warm[:], 0.0)
            wps = psS.tile([128, QT], F32, tag="s", name="wps")
            for _ in range(60):
                nc.tensor.matmul(
                    wps[:, 0:128], warm[:, 0:128], warm[:, 128:256],
                    start=True, stop=True,
                )

            for e in range(NE):
                sl = slice(e * SLAB, (e + 1) * SLAB)
                xe = get_xe(e)
                if e == 0:
                    # Slab 0 is DMA-bandwidth-critical: interleave x^T blocks
                    # with the first-head-pair weight blocks in consumption
                    # order, and defer everything else (second head-pair, wv,
                    # slab-1 x^T, RoPE tables beyond slab 0) behind them.
                    for g in range(0, MC, 4):
                        nc.sync.dma_start(
                            xe[:, g:g + 4, :], io["xTp"][0, :, g:g + 4, :]
                        )
                        nc.sync.dma_start(
                            wq_sb[:, 0, g:g + 4, :], io["wqp"][0, :, g:g + 4, :]
                        )
                        nc.sync.dma_start(
                            wk_sb[:, 0, g:g + 4, :], io["wkp"][0, :, g:g + 4, :]
                        )
                    nc.sync.dma_start(packC[:, 0:SLAB], io["packC"][:, 0:SLAB])
                    nc.sync.dma_start(packS[:, 0:SLAB], io["packS"][:, 0:SLAB])
                    nc.sync.dma_start(tri_sb[:], io["tri"])
                    nc.sync.dma_start(ones_sb[:], io["ones"])
                    nc.sync.dma_start(wq_sb[:, 1], io["wqp"][1])
                    nc.sync.dma_start(wk_sb[:, 1], io["wkp"][1])
                    nc.sync.dma_start(wv_sb[:], io["wvp"])
                    nc.sync.dma_start(get_xe(1)[:], io["xTp"][1])
                    nc.sync.dma_start(packC[:, SLAB:], io["packC"][:, SLAB:])
                    nc.sync.dma_start(packS[:, SLAB:], io["packS"][:, SLAB:])
                elif e < NE - 1:
                    nc.sync.dma_start(get_xe(e + 1)[:], io["xTp"][e + 1])

                # ---------- Phase A(e): Q/K head-pairs, then V directly in
                # [s, n] layout. Slab 0 fuses the q and k m-loops (4 matmuls
                # per x^T chunk) to halve the DMA consumption rate; later
                # slabs keep q/k sequential so each pair's RoPE eviction
                # hides under the next pair's matmuls. ----------
                def rope_evict(ps, dst, h):
                    # quadrant-16 RoPE: out = ps*packC + shuffle16(ps*packS)
                    t1 = ropep.tile([128, SLAB], F32, tag="t1", name="t1")
                    t2 = ropep.tile([128, SLAB], F32, tag="t2", name="t2")
                    t2s = ropep.tile([128, SLAB], F32, tag="t2s", name="t2s")
                    nc.vector.tensor_mul(t1[:], ps[:], packC[:, sl])
                    nc.vector.tensor_mul(t2[:], ps[:], packS[:, sl])
                    nc.vector.stream_shuffle(t2s[:], t2[:], SWAP16)
                    nc.vector.tensor_add(dst[:, h, sl], t1[:], t2s[:])

                if e == 0:
                    for hp in range(2):
                        hs = (2 * hp, 2 * hp + 1)
                        groups = [(wq_sb, qT_sb), (wk_sb, kT_sb)]
                        ps = {
                            (gi, h): psA.tile([128, SLAB], F32, tag="pa",
                                              name=f"pa{gi}{h}")
                            for gi in range(2) for h in hs
                        }
                        for m in range(MC):
                            for gi, (wsb, _) in enumerate(groups):
                                for h in hs:
                                    nc.tensor.matmul(
                                        ps[(gi, h)][:],
                                        wsb[:, hp, m,
                                            (h % 2) * D:(h % 2 + 1) * D],
                                        xe[:, m, :],
                                        start=(m == 0),
                                        stop=(m == MC - 1),
                                    )
                        for gi, (_, dst) in enumerate(groups):
                            for h in hs:
                                rope_evict(ps[(gi, h)], dst, h)
                else:
                    for wsb, dst, hp in (
                        (wq_sb, qT_sb, 0),
                        (wk_sb, kT_sb, 0),
                        (wq_sb, qT_sb, 1),
                        (wk_sb, kT_sb, 1),
                    ):
                        hs = (2 * hp, 2 * hp + 1)
                        ps = {
                            h: psA.tile([128, SLAB], F32, tag="pa",
                                        name=f"pa{h}")
                            for h in hs
                        }
                        for m in range(MC):
                            for h in hs:
                                nc.tensor.matmul(
                                    ps[h][:],
                                    wsb[:, hp, m, (h % 2) * D:(h % 2 + 1) * D],
                                    xe[:, m, :],
                                    start=(m == 0),
                                    stop=(m == MC - 1),
                                )
                        for h in hs:
                            rope_evict(ps[h], dst, h)

                for st in range(SLAB // 128):
                    psv = psA.tile([128, NH * D], F32, tag="pa", name="psv")
                    for m in range(MC):
                        nc.tensor.matmul(
                            psv[:],
                            xe[:, m, st * 128:(st + 1) * 128],
                            wv_sb[:, m, :],
                            start=(m == 0),
                            stop=(m == MC - 1),
                        )
                    gst = e * (SLAB // 128) + st
                    nc.scalar.copy(v_sb[:, gst, :], psv[:])

                if e < NE - 1:
                    emit_b(e, psS, psO, psD, False)

        # Last round: phase A is done, so its PSUM banks are free - use
        # 2-bank score tiles with one exp per key-tile pair so the scalar
        # engine keeps up with the PE in this exposed tail.
        with (
            tc.tile_pool(name="psS2", bufs=2, space="PSUM") as psS2,
            tc.tile_pool(name="psO2", bufs=2, space="PSUM") as psO2,
            tc.tile_pool(name="psD2", bufs=2, space="PSUM") as psD2,
        ):
            emit_b(NE - 1, psS2, psO2, psD2, True)


_NC_CACHE = {}


def _get_nc():
    if "nc" not in _NC_CACHE:
        nc = bacc.Bacc(
            "TRN2", target_bir_lowering=False, debug=False, num_devices=NCORES
        )
        io = {}
        for name, shape, dt_ in (
            ("xTp", [NE, 128, MC, SLAB], BF16),
            ("wqp", [2, 128, MC, 2 * D], BF16),
            ("wkp", [2, 128, MC, 2 * D], BF16),
            ("wvp", [128, MC, NH * D], BF16),
            ("packC", [128, S], BF16),
            ("packS", [128, S], BF16),
            ("tri", [128, 128], BF16),
            ("ones", [128, 128], BF16),
        ):
            io[name] = nc.dram_tensor(name, shape, dt_, kind="ExternalInput").ap()
        io["outT"] = nc.dram_tensor(
            "outT", [NH, 128, S], F32, kind="ExternalOutput"
        ).ap()
        with tile.TileContext(nc) as tc:
            build_attention(tc, io)
        nc.compile()
        _NC_CACHE["nc"] = nc
    return _NC_CACHE["nc"]


def kernel(x, Wq, Wk, Wv):
    x = np.asarray(x, dtype=np.float32)
    Wq = np.asarray(Wq, dtype=np.float32)
    Wk = np.asarray(Wk, dtype=np.float32)
    Wv = np.asarray(Wv, dtype=np.float32)

    nc = _get_nc()
    in_maps = [prep_core_inputs(x, Wq, Wk, Wv, c, S, M) for c in range(NCORES)]
    res = bass_utils.run_bass_kernel_spmd(nc, in_maps, core_ids=list(range(NCORES)))

    out = np.empty((2, S, M), dtype=np.float32)
    for c in range(NCORES):
        outT = res.results[c]["outT"]
        b, hg = c // 4, c % 4
        for h in range(NH):
            col = hg * NH * D + h * D
            out[b, :, col:col + D] = outT[h].T
    return out


# revision 30
# speedup vs baseline: 1.0310x; 1.0310x over previous
"""Trainium2 Bass kernel for causal multi-head attention with RoPE.

nn_CausalAttention: x [2, 2048, 2048], Wq/Wk/Wv [2048, 2048] (y = x @ W.T),
16 heads of dim 128, RoPE, causal fp32 softmax.

Sharding: core = (batch, head-group-of-4). Each of the 8 NeuronCores owns
one batch element and 4 heads (a 512-wide slice of the QKV output dim). The
full output is assembled on host by concatenating per-core feature slices (no
collectives needed).

Per-core kernel (Bass/Tile, bf16 matmul operands / fp32 accumulation+softmax,
measured end-to-end rel err ~4e-3 vs the 2e-2 gate; fp8 was measured to
break the gate and bf16 streams the PE at the same 1 col/cycle as fp32r):
  Fused pipeline over 4 sequence slabs of 512. For slab e:
    Phase A(e): q^T/k^T in [head_dim x seq] layout (weights stationary, x^T
      moving 512-wide); RoPE fused into the PSUM->SBUF eviction via the
      quadrant-16 row-permuted weight layout (pair-combine is one DVE
      stream_shuffle). v is projected directly into [seq x head_dim] layout
      (x^T tiles stationary, Wv^T moving) - no PE transposes; the psum
      eviction cast runs on the otherwise-idle scalar engine.
    Phase B(e): causal attention for query tile qt=e of all 4 heads in
      transposed-score layout S^T = K-tile^T^T @ q^T, diagonal tiles
      emitted first so accumulation-group start flags land on full-width
      matmuls and round-end drains wait on long-finished exps. exp on the
      scalar engine with the 1/sqrt(d) scale fused; diagonal tiles
      sub-ranged with a 128x128 triangular mask post-exp. Softmax
      denominator: full key tiles pair- then quad-summed on DVE (bf16),
      one ones-matmul per quad broadcasts the cross-partition sum into
      PSUM (1/4 the PE work of per-tile den matmuls). Normalization fused
      into the eviction with a fast approximate reciprocal, split in
      half-tiles to pipeline recip->mul->DMA; output is head-dim-major,
      untransposed on the host during the gather.
  Emitting B(e) between A(e) and A(e+1) hides the exp stream and DVE work
  under the next slab's projection matmuls. The last round (no slab behind
  it) switches to 2-bank score tiles - phase A's PSUM banks are free by
  then - with one exp per key-tile pair so the scalar engine outruns the PE.
Startup: all inputs are host-repacked partition-major so every DMA reads
multi-KB contiguous lines; slab-0 interleaves x/W blocks in consumption
order; ~50 dummy matmuls on a memset tile warm the PE clock-gate (HAM)
during the initial DMA window.
"""

import math

import numpy as np
import ml_dtypes

import concourse.bacc as bacc
import concourse.bass as bass
import concourse.mybir as mybir
import concourse.tile as tile
from concourse import bass_utils

F32 = mybir.dt.float32
BF16 = mybir.dt.bfloat16
AF = mybir.ActivationFunctionType

S = 2048
M = 2048
NCORES = 8

D = 128          # head dim
NH = 4           # heads per core
SLAB = 512       # sequence slab (phase A) == query tile (phase B)
NE = S // SLAB
MC = M // 128
QT = SLAB


def _rope_perm(n):
    """Row permutation for the quadrant-16 RoPE layout.

    New row p (within a 128-row head block): quadrant qd = p//32, r = p%32.
    r < 16  -> even element of pair i = 16*qd + r      (old row 2i)
    r >= 16 -> odd  element of pair i = 16*qd + (r-16) (old row 2i+1)
    Pair elements are 16 partitions apart inside one 32-partition quadrant,
    so the RoPE combine is a stream_shuffle with a 16-rotation mask.
    """
    perm = []
    for hb in range(n // D):
        base = hb * D
        for qd in range(4):
            perm += [base + 2 * (16 * qd + r) for r in range(16)]
            perm += [base + 2 * (16 * qd + r) + 1 for r in range(16)]
    return np.array(perm)


SWAP16 = [(i + 16) % 32 for i in range(32)]


def prep_core_inputs(x, Wq, Wk, Wv, core, S, M):
    """Host-side shard prep for one core: batch b = core//4, heads
    [hg*4, hg*4+4) where hg = core%4 (rows [hg*512, hg*512+512) of W*)."""
    b, hg = core // 4, core % 4
    nsl = slice(hg * NH * D, (hg + 1) * NH * D)
    perm = _rope_perm(NH * D)
    wq = Wq[nsl][perm]
    wk = Wk[nsl][perm]
    wv = Wv[nsl]

    def bf(a):
        return np.ascontiguousarray(a).astype(ml_dtypes.bfloat16)

    # Partition-major repacks so every DMA reads large contiguous lines
    # per partition (host prep is not on the measured clock):
    #   xTp[e, p, mo, s] = x[b].T[mo*128+p, e*SLAB+s]
    #   w*p[hp, p, mo, c] = W*.T[mo*128+p, hp*256+c]   (head-pair-major)
    #   wvp[p, mo, c]    = Wv.T[mo*128+p, c]
    xT = x[b].T
    xTp = xT.reshape(MC, 128, NE, SLAB).transpose(2, 1, 0, 3)
    wqp = wq.T.reshape(MC, 128, 2, 2 * D).transpose(2, 1, 0, 3)
    wkp = wk.T.reshape(MC, 128, 2, 2 * D).transpose(2, 1, 0, 3)
    wvp = wv.T.reshape(MC, 128, NH * D).transpose(1, 0, 2)

    theta = np.exp(
        -np.float32(np.log(10000.0))
        * (np.arange(0, D, 2, dtype=np.float32) / np.float32(D))
    ).astype(np.float32)
    pos = np.arange(S, dtype=np.float32)
    freqs = theta[:, None] * pos[None, :]  # [64, S]
    cos_t, sin_t = np.cos(freqs), np.sin(freqs)
    # quadrant-16 layout: partition p -> pair i(p) = 16*(p//32) + (p%16)
    p = np.arange(128)
    i_of_p = 16 * (p // 32) + (p % 16)
    is_odd = (p % 32) >= 16
    packC = cos_t[i_of_p].astype(np.float32)                    # [128, S]
    packS = np.where(
        is_odd[:, None], -sin_t[i_of_p], sin_t[i_of_p]
    ).astype(np.float32)

    kk, qq = np.meshgrid(np.arange(128), np.arange(128), indexing="ij")
    tri = (kk <= qq).astype(np.float32)

    return {
        "xTp": bf(xTp),            # [NE, 128, MC, SLAB]
        "wqp": bf(wqp),            # [2, 128, MC, 2*D]
        "wkp": bf(wkp),
        "wvp": bf(wvp),            # [128, MC, NH*D]
        "packC": packC.astype(ml_dtypes.bfloat16),
        "packS": packS.astype(ml_dtypes.bfloat16),
        "tri": bf(tri),
        "ones": bf(np.ones((128, 128), dtype=np.float32)),
    }


def build_attention(tc: tile.TileContext, io: dict):
    nc = tc.nc
    scale = 1.0 / math.sqrt(D)
    outT = io["outT"]

    with (
        tc.tile_pool(name="wpool", bufs=1) as wpool,
        tc.tile_pool(name="qkvp", bufs=1) as qkvp,
        tc.tile_pool(name="packp", bufs=1) as packp,
        tc.tile_pool(name="constp", bufs=1) as constp,
        tc.tile_pool(name="xp", bufs=2) as xpool,
        tc.tile_pool(name="rope", bufs=2) as ropep,
        tc.tile_pool(name="expp", bufs=8) as expp,
        tc.tile_pool(name="dp", bufs=2) as dpool,
        tc.tile_pool(name="outp", bufs=2) as outp,
    ):
        wq_sb = wpool.tile([128, 2, MC, 2 * D], BF16, name="wq_sb")
        wk_sb = wpool.tile([128, 2, MC, 2 * D], BF16, name="wk_sb")
        wv_sb = wpool.tile([128, MC, NH * D], BF16, name="wv_sb")
        qT_sb = qkvp.tile([128, NH, S], BF16, name="qT_sb")
        kT_sb = qkvp.tile([128, NH, S], BF16, name="kT_sb")
        v_sb = qkvp.tile([128, S // 128, NH * D], BF16, name="v_sb")
        packC = packp.tile([128, S], BF16, name="packC")
        packS = packp.tile([128, S], BF16, name="packS")
        tri_sb = constp.tile([128, 128], BF16, name="tri_sb")
        ones_sb = constp.tile([128, 128], BF16, name="ones_sb")
        warm = constp.tile([128, 256], BF16, name="warm")

        xe_tiles = {}

        def get_xe(e):
            if e not in xe_tiles:
                xe_tiles[e] = xpool.tile(
                    [128, MC, SLAB], BF16, tag="xe", name=f"xe{e}"
                )
            return xe_tiles[e]

        def emit_b(e, psSp, psOp, psDp, paired):
            """Phase B round: causal attention for query tile qt=e, all
            heads, in transposed-score layout. paired=True packs two key
            tiles into one 2-bank PSUM score tile with a single exp per
            pair (used for the last round, where the exp stream would
            otherwise outpace the PE). The softmax denominator uses bf16
            pair- then quad-sums on DVE, with one ones-matmul per quad."""
            qt = e
            nkt = 4 * (e + 1)
            nfull = nkt - 4
            for h in range(NH):
                out_ps = psOp.tile([128, QT], F32, tag="o", name="out_ps")
                den_ps = psDp.tile([128, QT], F32, tag="d", name="den_ps")
                # start/stop by emission order (diagonal tiles go first, so
                # the first matmul of each accumulation group is full-width)
                den_left = [4 + (nfull // 4 if not paired else nfull // 4)]
                av_left = [nkt]
                pend = []
                dpairs = []

                def dmm(src, rs, _unused=None):
                    nc.tensor.matmul(
                        den_ps[:, rs:], ones_sb[:], src,
                        start=(den_left[0] == 4 + nfull // 4),
                        stop=(den_left[0] == 1),
                    )
                    den_left[0] -= 1

                def av(src, kt, rs):
                    nc.tensor.matmul(
                        out_ps[:, rs:], v_sb[:, kt, h * D:(h + 1) * D], src,
                        start=(av_left[0] == nkt), stop=(av_left[0] == 1),
                    )
                    av_left[0] -= 1

                def drain(n):
                    while len(pend) > n:
                        for f in pend.pop(0):
                            f()

                def add_pair(src0, src1, ops):
                    dpair = dpool.tile([128, QT], BF16, tag="dp",
                                       name="dpair")
                    nc.vector.tensor_add(dpair[:], src0, src1)
                    dpairs.append(dpair)
                    if len(dpairs) == 2:
                        a, b_ = dpairs
                        dpairs.clear()
                        dq = dpool.tile([128, QT], BF16, tag="dq",
                                        name="dquad")
                        nc.vector.tensor_add(dq[:], a[:], b_[:])
                        ops.append(lambda dq=dq: dmm(dq[:], 0))

                if not paired:
                    held = {}
                    for kt in list(range(nfull, nkt)) + list(range(nfull)):
                        j = kt - nfull
                        rs = 128 * j if j > 0 else 0
                        s_ps = psSp.tile([128, QT], F32, tag="s", name="s_ps")
                        nc.tensor.matmul(
                            s_ps[:, rs:],
                            kT_sb[:, h, kt * 128:(kt + 1) * 128],
                            qT_sb[:, h, qt * QT + rs:(qt + 1) * QT],
                            start=True, stop=True,
                        )
                        expS = expp.tile([128, QT], BF16, tag="exp",
                                         name="expS")
                        nc.scalar.activation(
                            expS[:, rs:], s_ps[:, rs:], AF.Exp, scale=scale
                        )
                        if j >= 0:
                            nc.vector.tensor_mul(
                                expS[:, 128 * j:128 * (j + 1)],
                                expS[:, 128 * j:128 * (j + 1)],
                                tri_sb[:],
                            )
                        ops = []
                        if kt < nfull:
                            if kt % 2 == 0:
                                held[kt] = expS
                            else:
                                add_pair(held.pop(kt - 1)[:], expS[:], ops)
                        else:
                            ops.append(
                                lambda expS=expS, rs=rs, kt=kt:
                                dmm(expS[:, rs:], rs)
                            )
                        ops.append(
                            lambda expS=expS, kt=kt, rs=rs:
                            av(expS[:, rs:], kt, rs)
                        )
                        pend.append(ops)
                        drain(3)
                    drain(0)
                else:
                    npair = nkt // 2
                    for p in list(range(nfull // 2, npair)) + list(range(nfull // 2)):
                        k0, k1 = 2 * p, 2 * p + 1
                        j0, j1 = k0 - nfull, k1 - nfull
                        rs0 = 128 * j0 if j0 > 0 else 0
                        rs1 = 128 * j1 if j1 > 0 else 0
                        sp = psSp.tile([128, 2 * QT], F32, tag="s2",
                                       name="sp")
                        nc.tensor.matmul(
                            sp[:, rs0:QT],
                            kT_sb[:, h, k0 * 128:(k0 + 1) * 128],
                            qT_sb[:, h, qt * QT + rs0:(qt + 1) * QT],
                            start=True, stop=True,
                        )
                        nc.tensor.matmul(
                            sp[:, QT + rs1:],
                            kT_sb[:, h, k1 * 128:(k1 + 1) * 128],
                            qT_sb[:, h, qt * QT + rs1:(qt + 1) * QT],
                            start=True, stop=True,
                        )
                        expS = expp.tile([128, 2 * QT], BF16, tag="exp2",
                                         name="expS2")
                        if j1 <= 0:
                            nc.scalar.activation(
                                expS[:, rs0:], sp[:, rs0:], AF.Exp,
                                scale=scale,
                            )
                        else:
                            # diagonal pair: skip the stale sub-range the
                            # second score matmul left unwritten
                            nc.scalar.activation(
                                expS[:, rs0:QT], sp[:, rs0:QT], AF.Exp,
                                scale=scale,
                            )
                            nc.scalar.activation(
                                expS[:, QT + rs1:], sp[:, QT + rs1:], AF.Exp,
                                scale=scale,
                            )
                        for i, j in ((0, j0), (1, j1)):
                            if j >= 0:
                                cs = slice(i * QT + 128 * j,
                                           i * QT + 128 * (j + 1))
                                nc.vector.tensor_mul(
                                    expS[:, cs], expS[:, cs], tri_sb[:]
                                )
                        ops = []
                        if k1 < nfull:
                            add_pair(expS[:, 0:QT], expS[:, QT:], ops)
                        else:
                            ops.append(
                                lambda expS=expS, rs0=rs0:
                                dmm(expS[:, rs0:QT], rs0)
                            )
                            ops.append(
                                lambda expS=expS, rs1=rs1:
                                dmm(expS[:, QT + rs1:], rs1)
                            )
                        ops.append(
                            lambda expS=expS, k0=k0, rs0=rs0:
                            av(expS[:, rs0:QT], k0, rs0)
                        )
                        ops.append(
                            lambda expS=expS, k1=k1, rs1=rs1:
                            av(expS[:, QT + rs1:], k1, rs1)
                        )
                        pend.append(ops)
                        drain(2)
                    drain(0)

                # normalize+store in two half-tiles so the recip->mul->DMA
                # chain pipelines (shortens the end-of-kernel drain)
                for hf in range(2):
                    cs = slice(hf * (QT // 2), (hf + 1) * (QT // 2))
                    recip = outp.tile([128, QT // 2], F32, tag="recip",
                                      name="recip")
                    nc.vector.reciprocal_approx_fast(recip[:], den_ps[:, cs])
                    o_sb = outp.tile([128, QT // 2], F32, tag="o",
                                     name="o_sb")
                    nc.vector.tensor_mul(o_sb[:], out_ps[:, cs], recip[:])
                    nc.sync.dma_start(
                        outT[h, :, qt * QT + hf * (QT // 2):
                             qt * QT + (hf + 1) * (QT // 2)],
                        o_sb[:],
                    )

        with (
            tc.tile_pool(name="psA", bufs=4, space="PSUM") as psA,
            tc.tile_pool(name="psS", bufs=2, space="PSUM") as psS,
            tc.tile_pool(name="psO", bufs=1, space="PSUM") as psO,
            tc.tile_pool(name="psD", bufs=1, space="PSUM") as psD,
        ):
            # HAM prewarm: the PE clock-gate needs ~3.4us of sustained
            # activity to unthrottle from 1.2 to 2.4 GHz. The first input
            # DMAs take ~10us to land, so spend that dead window on dummy
            # matmuls over a memset tile (no DMA dependency) and the real
            # matmuls start at full clock.
            nc.vector.memset(warm[:], 0.0)
            wps = psS.tile([128, QT], F32, tag="s", name="wps")
            for _ in range(60):
                nc.tensor.matmul(
                    wps[:, 0:128], warm[:, 0:128], warm[:, 128:256],
                    start=True, stop=True,
                )

            for e in range(NE):
                sl = slice(e * SLAB, (e + 1) * SLAB)
                xe = get_xe(e)
                if e == 0:
                    # Slab 0 is DMA-bandwidth-critical: interleave x^T blocks
                    # with the first-head-pair weight blocks in consumption
                    # order, and defer everything else (second head-pair, wv,
                    # slab-1 x^T, RoPE tables beyond slab 0) behind them.
                    for g in range(0, MC, 4):
                        nc.sync.dma_start(
                            xe[:, g:g + 4, :], io["xTp"][0, :, g:g + 4, :]
                        )
                        nc.sync.dma_start(
                            wq_sb[:, 0, g:g + 4, :], io["wqp"][0, :, g:g + 4, :]
                        )
                        nc.sync.dma_start(
                            wk_sb[:, 0, g:g + 4, :], io["wkp"][0, :, g:g + 4, :]
                        )
                    nc.sync.dma_start(packC[:, 0:SLAB], io["packC"][:, 0:SLAB])
                    nc.sync.dma_start(packS[:, 0:SLAB], io["packS"][:, 0:SLAB])
                    nc.sync.dma_start(tri_sb[:], io["tri"])
                    nc.sync.dma_start(ones_sb[:], io["ones"])
                    nc.sync.dma_start(wq_sb[:, 1], io["wqp"][1])
                    nc.sync.dma_start(wk_sb[:, 1], io["wkp"][1])
                    nc.sync.dma_start(wv_sb[:], io["wvp"])
                    nc.sync.dma_start(get_xe(1)[:], io["xTp"][1])
                    nc.sync.dma_start(packC[:, SLAB:], io["packC"][:, SLAB:])
                    nc.sync.dma_start(packS[:, SLAB:], io["packS"][:, SLAB:])
                elif e < NE - 1:
                    nc.sync.dma_start(get_xe(e + 1)[:], io["xTp"][e + 1])

                # ---------- Phase A(e): Q/K head-pairs, then V directly in
                # [s, n] layout. Slab 0 fuses the q and k m-loops (4 matmuls
                # per x^T chunk) to halve the DMA consumption rate; later
                # slabs keep q/k sequential so each pair's RoPE eviction
                # hides under the next pair's matmuls. ----------
                def rope_evict(ps, dst, h):
                    # quadrant-16 RoPE: out = ps*packC + shuffle16(ps*packS)
                    t1 = ropep.tile([128, SLAB], F32, tag="t1", name="t1")
                    t2 = ropep.tile([128, SLAB], F32, tag="t2", name="t2")
                    t2s = ropep.tile([128, SLAB], F32, tag="t2s", name="t2s")
                    nc.vector.tensor_mul(t1[:], ps[:], packC[:, sl])
                    nc.vector.tensor_mul(t2[:], ps[:], packS[:, sl])
                    nc.vector.stream_shuffle(t2s[:], t2[:], SWAP16)
                    nc.vector.tensor_add(dst[:, h, sl], t1[:], t2s[:])

                if e == 0:
                    for hp in range(2):
                        hs = (2 * hp, 2 * hp + 1)
                        groups = [(wq_sb, qT_sb), (wk_sb, kT_sb)]
                        ps = {
                            (gi, h): psA.tile([128, SLAB], F32, tag="pa",
                                              name=f"pa{gi}{h}")
                            for gi in range(2) for h in hs
                        }
                        for m in range(MC):
                            for gi, (wsb, _) in enumerate(groups):
                                for h in hs:
                                    nc.tensor.matmul(
                                        ps[(gi, h)][:],
                                        wsb[:, hp, m,
                                            (h % 2) * D:(h % 2 + 1) * D],
                                        xe[:, m, :],
                                        start=(m == 0),
                                        stop=(m == MC - 1),
                                    )
                        for gi, (_, dst) in enumerate(groups):
                            for h in hs:
                                rope_evict(ps[(gi, h)], dst, h)
                else:
                    for wsb, dst, hp in (
                        (wq_sb, qT_sb, 0),
                        (wk_sb, kT_sb, 0),
                        (wq_sb, qT_sb, 1),
                        (wk_sb, kT_sb, 1),
                    ):
                        hs = (2 * hp, 2 * hp + 1)
                        ps = {
                            h: psA.tile([128, SLAB], F32, tag="pa",
                                        name=f"pa{h}")
                            for h in hs
                        }
                        for m in range(MC):
                            for h in hs:
                                nc.tensor.matmul(
                                    ps[h][:],
                                    wsb[:, hp, m, (h % 2) * D:(h % 2 + 1) * D],
                                    xe[:, m, :],
                                    start=(m == 0),
                                    stop=(m == MC - 1),
                                )
                        for h in hs:
                            rope_evict(ps[h], dst, h)

                for st in range(SLAB // 128):
                    psv = psA.tile([128, NH * D], F32, tag="pa", name="psv")
                    for m in range(MC):
                        nc.tensor.matmul(
                            psv[:],
                            xe[:, m, st * 128:(st + 1) * 128],
                            wv_sb[:, m, :],
                            start=(m == 0),
                            stop=(m == MC - 1),
                        )
                    gst = e * (SLAB // 128) + st
                    nc.scalar.copy(v_sb[:, gst, :], psv[:])

                if e < NE - 1:
                    emit_b(e, psS, psO, psD, False)

        # Last round: phase A is done, so its PSUM banks are free - use
        # 2-bank score tiles with one exp per key-tile pair so the scalar
        # engine keeps up with the PE in this exposed tail.
        # pool-open order places the 2-bank score tiles on banks that
        # have been idle since B2 (not on slab-3's just-evicted psv banks)
        with (
            tc.tile_pool(name="psO2", bufs=2, space="PSUM") as psO2,
            tc.tile_pool(name="psD2", bufs=2, space="PSUM") as psD2,
            tc.tile_pool(name="psS2", bufs=2, space="PSUM") as psS2,
        ):
            emit_b(NE - 1, psS2, psO2, psD2, True)


_NC_CACHE = {}


def _get_nc():
    if "nc" not in _NC_CACHE:
        nc = bacc.Bacc(
            "TRN2", target_bir_lowering=False, debug=False, num_devices=NCORES
        )
        io = {}
        for name, shape, dt_ in (
            ("xTp", [NE, 128, MC, SLAB], BF16),
            ("wqp", [2, 128, MC, 2 * D], BF16),
            ("wkp", [2, 128, MC, 2 * D], BF16),
            ("wvp", [128, MC, NH * D], BF16),
            ("packC", [128, S], BF16),
            ("packS", [128, S], BF16),
            ("tri", [128, 128], BF16),
            ("ones", [128, 128], BF16),
        ):
            io[name] = nc.dram_tensor(name, shape, dt_, kind="ExternalInput").ap()
        io["outT"] = nc.dram_tensor(
            "outT", [NH, 128, S], F32, kind="ExternalOutput"
        ).ap()
        with tile.TileContext(nc) as tc:
            build_attention(tc, io)
        nc.compile()
        _NC_CACHE["nc"] = nc
    return _NC_CACHE["nc"]


def kernel(x, Wq, Wk, Wv):
    x = np.asarray(x, dtype=np.float32)
    Wq = np.asarray(Wq, dtype=np.float32)
    Wk = np.asarray(Wk, dtype=np.float32)
    Wv = np.asarray(Wv, dtype=np.float32)

    nc = _get_nc()
    in_maps = [prep_core_inputs(x, Wq, Wk, Wv, c, S, M) for c in range(NCORES)]

    # retry once on a non-finite result (transient device corruption)
    for _attempt in range(2):
        res = bass_utils.run_bass_kernel_spmd(
            nc, in_maps, core_ids=list(range(NCORES))
        )
        out = np.empty((2, S, M), dtype=np.float32)
        for c in range(NCORES):
            outT = res.results[c]["outT"]
            b, hg = c // 4, c % 4
            for h in range(NH):
                col = hg * NH * D + h * D
                out[b, :, col:col + D] = outT[h].T
        if np.isfinite(out).all():
            break
    return out
